# revision 1
# baseline (speedup 1.0000x reference)
"""Trainium2 Bass kernel for nn_Encoder_60112362275055 (GRU with skip connections).

B=64, T=512, X=256, H=1024, skip_size=5. Output = 2 * h_{T-1}  -> [64, 1024].

Data-parallel over batch (8 cores x B_local=8), zero cross-core traffic.
The skip structure (w1[t]==0 drops the h_{t-1} dependency) turns the scan
into a DAG of ~88 topological levels (avg width ~6); all nodes in a level
are batched into one [M=w*8, 1024] x [1024, 3072] matmul per core.

v2 design (vs the fp32r baseline):
- all matmul operands and the elementwise tail run in bf16 (psum stays f32);
  measured numerics: rel_err ~6e-3 against the f32 reference (gate 2e-2).
- the 16 PE-mode transposes per level are replaced by DMA x-bar transposes
  (h_blend^T -> batch layout, h_new -> hist^T), keeping the tensor engine
  free for the W_hh stream and HAM-warm at 2.4 GHz.
- per level the 48 gate matmuls are split into two bank-halves
  (r0,n0,z0 then r1,n1,z1, chunk-major); the elementwise tail of each half
  runs in 256-col quarters and hides under the other half / the next
  level's stream.
"""

import sys

import numpy as np

sys.path.insert(0, "/opt/trn_rl_repo")

import ml_dtypes

import concourse.bacc as bacc
import concourse.mybir as mybir
from concourse import tile
from concourse.bass_utils import run_bass_kernel_spmd

B, T, X, H = 64, 512, 256, 1024
SKIP = 5
NCORES = 8
BL = B // NCORES  # 8
G3 = 3 * H  # 3072
NB = G3 // 512  # 6 psum banks per level
KC = H // 128  # 8 K-chunks
NLEVH = 16  # rolling history depth in levels (max dep distance is 10)
WCOL = 80  # padded node-cols per (level, chunk) hist region (max M=72 -> 80)

f32 = mybir.dt.float32
bf16 = mybir.dt.bfloat16
AF = mybir.ActivationFunctionType
bfnp = ml_dtypes.bfloat16


def _skip_plan(T, skip_size):
    slots = np.zeros(T, np.int32)
    use_zero = np.zeros(T, np.float32)
    for i in range(T):
        if i < skip_size:
            if 2 * i < skip_size:
                use_zero[i] = 1.0
            else:
                slots[i] = (skip_size - i) - 1
        else:
            if i - skip_size < skip_size:
                use_zero[i] = 1.0
            else:
                slots[i] = 2 * skip_size - 1
    return slots, use_zero


def _plan(w1, w2):
    """Topological levels + per-node dependency spec (level, index) pairs."""
    slots, use_zero = _skip_plan(T, SKIP)
    d1 = np.full(T, -1, np.int64)
    d2 = np.full(T, -1, np.int64)
    for t in range(T):
        if w1[t] == 1 and t - 1 >= 0:
            d1[t] = t - 1
        if w2[t] == 1 and use_zero[t] == 0.0 and t - 1 - slots[t] >= 0:
            d2[t] = t - 1 - slots[t]
    level = np.zeros(T, np.int64)
    for t in range(T):
        deps = [d for d in (d1[t], d2[t]) if d >= 0]
        level[t] = 1 + max((level[d] for d in deps), default=-1)
    nlev = int(level.max()) + 1
    levels = [[] for _ in range(nlev)]
    idx_of = {}
    for t in range(T):
        idx_of[t] = len(levels[level[t]])
        levels[level[t]].append(t)
    order = [t for lv in levels for t in lv]
    # max dep distance must fit the rolling history window
    maxdist = 0
    for t in range(T):
        for d in (d1[t], d2[t]):
            if d >= 0:
                maxdist = max(maxdist, int(level[t] - level[d]))
    assert maxdist + 3 <= NLEVH, (maxdist, "NLEVH too small")
    return levels, order, level, idx_of, d1, d2


def _build(levels, lv_of, idx_of, d1, d2):
    nc = bacc.Bacc(None)

    xs_d = nc.dram_tensor("xs", [2, 128, T * BL], bf16, kind="ExternalInput")
    wih_d = nc.dram_tensor("wih", [2, 128, G3], bf16, kind="ExternalInput")
    # per K-chunk: 3072 gate cols + 128 identity cols (h_blend passthrough)
    GW = G3 + 128
    whh_d = nc.dram_tensor("whh", [KC, 128, GW], bf16, kind="ExternalInput")
    biasg_d = nc.dram_tensor("biasg", [128, G3], f32, kind="ExternalInput")
    bias2_d = nc.dram_tensor("bias2", [128, H], bf16, kind="ExternalInput")
    ident_d = nc.dram_tensor("ident", [128, 128], bf16, kind="ExternalInput")
    out_d = nc.dram_tensor("out", [BL, H], f32, kind="ExternalOutput")
    xi_d = nc.dram_tensor("xi_scratch", [T * BL, G3], bf16)

    NLEV = len(levels)
    t_out = T - 1
    lv_out = int(lv_of[t_out])

    with tile.TileContext(nc) as tc:
        with (
            tc.tile_pool(name="rec", bufs=1) as rpool,
            tc.tile_pool(name="ps", bufs=8, space="PSUM") as pspool,
        ):
            # persistent tensors; whh DMA issued first so it hides under
            # phase-1 compute
            whh = rpool.tile([128, KC * GW], bf16)  # 50 KB/partition
            nc.sync.dma_start(
                whh[:].rearrange("p (k f) -> p k f", k=KC),
                whh_d.rearrange("k p f -> p k f"),
            )
            b2bc = rpool.tile([128, H], bf16)
            nc.sync.dma_start(b2bc[:], bias2_d[:])
            ident = rpool.tile([128, 128], bf16)
            nc.sync.dma_start(ident[:], ident_d[:])
            # hist^T: [128, level-slot(16) x chunk(8) x node-col(80)] bf16
            hist = rpool.tile([128, NLEVH * KC * WCOL], bf16)

            # ---------- phase 1: xi = x @ W_ih.T + bias (level-sorted rows) ----
            with tc.tile_pool(name="ph1", bufs=1) as p1, tc.tile_pool(
                name="ph1b", bufs=3
            ) as p1b:
                wih = p1.tile([128, 2 * G3], bf16)
                nc.sync.dma_start(
                    wih[:].rearrange("p (k f) -> p k f", k=2),
                    wih_d.rearrange("k p f -> p k f"),
                )
                xs = p1.tile([128, 2 * T * BL], bf16)
                nc.sync.dma_start(
                    xs[:].rearrange("p (k f) -> p k f", k=2),
                    xs_d.rearrange("k p f -> p k f"),
                )
                biasg = p1.tile([128, G3], f32)
                nc.sync.dma_start(biasg[:], biasg_d[:])
                MT = T * BL // 128  # 32 M-tiles
                for m in range(MT):
                    for nb in range(NB):
                        ps = pspool.tile([128, 512], f32, tag="ps")
                        for k in range(2):
                            nc.tensor.matmul(
                                ps[:],
                                xs[:, k * T * BL + m * 128 : k * T * BL + (m + 1) * 128],
                                wih[:, k * G3 + nb * 512 : k * G3 + (nb + 1) * 512],
                                start=(k == 0),
                                stop=(k == 1),
                            )
                        sb = p1b.tile([128, 512], bf16, tag="xsb")
                        nc.vector.tensor_add(
                            sb[:], ps[:], biasg[:, nb * 512 : (nb + 1) * 512]
                        )
                        nc.sync.dma_start(
                            xi_d[m * 128 : (m + 1) * 128, nb * 512 : (nb + 1) * 512],
                            sb[:],
                        )

            # ---------- phase 2: recurrence over DAG levels ----------
            with (
                tc.tile_pool(name="hblp", bufs=3) as hblp,
                tc.tile_pool(name="rzp", bufs=2) as rzp,
                tc.tile_pool(name="npp", bufs=2) as npp,
                tc.tile_pool(name="hnp", bufs=2) as hnp,
                tc.tile_pool(name="xiin", bufs=3) as xipool,
                tc.tile_pool(name="outp", bufs=1) as opool,
            ):

                def hist_dep(t, c0, cn):
                    """[128, cn, 8] view of dep node t's h^T, chunks c0..c0+cn."""
                    sl = int(lv_of[t]) % NLEVH
                    j = idx_of[t]
                    return hist[:].rearrange(
                        "p (s c j) -> p s c j", s=NLEVH, c=KC
                    )[:, sl, c0 : c0 + cn, j * BL : (j + 1) * BL]

                def emit_blend(eng, hbl, nodes, half):
                    """Blend deps into hbl chunks [4*half .. 4*half+4)."""
                    c0 = 4 * half
                    hv = hbl[:].rearrange("p (c f) -> p c f", c=KC)
                    for i, t in enumerate(nodes):
                        dst = hv[:, c0 : c0 + 4, i * BL : (i + 1) * BL]
                        a, b2 = int(d1[t]), int(d2[t])
                        if a < 0 and b2 < 0:
                            eng.memset(dst, 0.0)
                        elif a >= 0 and b2 >= 0:
                            eng.tensor_add(dst, hist_dep(a, c0, 4), hist_dep(b2, c0, 4))
                        else:
                            eng.tensor_copy(dst, hist_dep(a if a >= 0 else b2, c0, 4))

                def emit_pt(pt, chunks, M, hnew):
                    """PE-transpose h_new chunks into psum (batch -> hist^T)."""
                    for i, c in enumerate(chunks):
                        nc.tensor.transpose(
                            pt[:, i * 128 : i * 128 + M],
                            hnew[:M, c * 128 : (c + 1) * 128],
                            ident[:M, :M],
                        )

                def emit_hist_copy(pt, chunks, M, sl):
                    for i, c in enumerate(chunks):
                        nc.vector.tensor_copy(
                            hist[:, sl * KC * WCOL + c * WCOL : sl * KC * WCOL + c * WCOL + M],
                            pt[:, i * 128 : i * 128 + M],
                        )

                def emit_quarter(q, M, psb, xi, rz, np_, hnew):
                    """Tail for cols [q*256, q*256+256): gates + h_new."""
                    qs = slice(q * 256, (q + 1) * 256)
                    half, qc = q // 2, (q % 2) * 256
                    pcs = slice(qc, qc + 256)
                    ps_r, ps_n, ps_z = psb[half], psb[2 + half], psb[4 + half]
                    ps_hb = psb[6 + half]  # h_blend passthrough (batch layout)
                    zs = slice(H + q * 256, H + (q + 1) * 256)
                    # r = sigmoid(hr + xr)   (head of the n-chain -> first)
                    nc.vector.tensor_add(rz[:M, qs], ps_r[:M, pcs], xi[:M, qs])
                    nc.scalar.activation(rz[:M, qs], rz[:M, qs], AF.Sigmoid)
                    # z = sigmoid(hz + xz)   (independent, fills ACT/DVE slack)
                    nc.vector.tensor_add(
                        rz[:M, zs], ps_z[:M, pcs], xi[:M, 2 * H + q * 256 : 2 * H + (q + 1) * 256]
                    )
                    nc.scalar.activation(rz[:M, zs], rz[:M, zs], AF.Sigmoid)
                    # n = tanh(xn + r * psum_n)   (b_hh_n was matmul-folded)
                    nc.vector.tensor_mul(np_[:M, qs], ps_n[:M, pcs], rz[:M, qs])
                    nc.vector.tensor_add(
                        np_[:M, qs], np_[:M, qs], xi[:M, H + q * 256 : H + (q + 1) * 256]
                    )
                    nc.scalar.activation(np_[:M, qs], np_[:M, qs], AF.Tanh)
                    # h_new = n + z*(hb - n)
                    nc.vector.tensor_sub(hnew[:M, qs], ps_hb[:M, pcs], np_[:M, qs])
                    nc.vector.tensor_mul(hnew[:M, qs], hnew[:M, qs], rz[:M, zs])
                    nc.vector.tensor_add(hnew[:M, qs], hnew[:M, qs], np_[:M, qs])

                # row offset of each level's xi rows (level-sorted, tight)
                srow = [0]
                for lvw in levels:
                    srow.append(srow[-1] + len(lvw))

                def new_hbl(lv):
                    return hblp.tile([128, KC * 128], bf16, tag="hbl", name="hbl")

                def gate_mms(hbl, psb, banks, with_id):
                    for c in range(KC):
                        lhsT = hbl[:, c * 128 : (c + 1) * 128]
                        for nb in banks:
                            nc.tensor.matmul(
                                psb[nb][:, :],
                                lhsT,
                                whh[:, c * GW + nb * 512 : c * GW + (nb + 1) * 512],
                                start=(c == 0),
                                stop=(c == KC - 1),
                            )
                            if c == 0 and nb in (2, 3):
                                # fold b_hh_n into the n-psum via a ones-row MM
                                nc.tensor.matmul(
                                    psb[nb][:, :],
                                    ones1[0:1, :],
                                    b2bc[0:1, (nb - 2) * 512 : (nb - 1) * 512],
                                    start=False,
                                    stop=False,
                                )
                        if with_id:
                            nc.tensor.matmul(
                                psb[6 + c // 4][:, (c % 4) * 128 : (c % 4) * 128 + 128],
                                lhsT,
                                whh[:, c * GW + G3 : c * GW + G3 + 128],
                                start=True,
                                stop=True,
                            )

                ones1 = rpool.tile([1, 128], bf16)
                nc.vector.memset(ones1[:], 1.0)

                # pre-loop: blend of level 0 (both halves)
                hbl_cur = new_hbl(0)
                emit_blend(nc.vector, hbl_cur, levels[0], 0)
                emit_blend(nc.gpsimd, hbl_cur, levels[0], 1)

                pending = None  # (hnew, P16, sl) of the previous level's half B

                def gate_mms_halfA(hbl, psb, lv):
                    """Half A in two chunk-groups with the previous level's
                    ptB transposes + hist copies + this level's blend-B
                    interleaved between them (PE stays busy while the
                    previous tail-B finishes)."""
                    nonlocal pending
                    M = BL * len(levels[lv])
                    for c in range(KC):
                        lhsT = hbl[:, c * 128 : c * 128 + M]
                        for nb in (0, 2, 4):
                            nc.tensor.matmul(
                                psb[nb][:M, :],
                                lhsT,
                                whh[:, c * GW + nb * 512 : c * GW + (nb + 1) * 512],
                                start=(c == 0),
                                stop=(c == KC - 1),
                            )
                            if c == 0 and nb == 2:
                                nc.tensor.matmul(
                                    psb[nb][:M, :],
                                    ones1[0:1, :M],
                                    b2bc[0:1, 0:512],
                                    start=False,
                                    stop=False,
                                )
                        nc.tensor.matmul(
                            psb[6 + c // 4][:M, (c % 4) * 128 : (c % 4) * 128 + 128],
                            lhsT,
                            whh[:, c * GW + G3 : c * GW + G3 + 128],
                            start=True,
                            stop=True,
                        )
                        if c == 3 and pending is not None:
                            # previous level's h_new chunks 4..7 -> hist
                            p_hnew, p_M, p_sl = pending
                            ptB = pspool.tile([128, 512], bf16, tag="ps", name="ptB")
                            emit_pt(ptB, (4, 5, 6, 7), p_M, p_hnew)
                            emit_hist_copy(ptB, (4, 5, 6, 7), p_M, p_sl)
                            pending = None
                            # blend-B of THIS level (needs those hist chunks)
                            emit_blend(nc.gpsimd, hbl, levels[lv], 1)

                for lv, nodes in enumerate(levels):
                    w = len(nodes)
                    M = BL * w
                    P16 = ((M + 15) // 16) * 16
                    sl = lv % NLEVH
                    hbl = hbl_cur

                    # xi rows for this level (SWDGE, off the HWDGE queues)
                    xi = xipool.tile([128, G3], bf16, tag="xi")
                    nc.gpsimd.dma_start(
                        xi[:M, :], xi_d[srow[lv] * BL : srow[lv] * BL + M, :]
                    )

                    psb = [None] * (NB + 2)
                    for nb in (0, 2, 4, 6, 7):
                        psb[nb] = pspool.tile([128, 512], f32, tag="ps", name=f"ps{nb}")
                    gate_mms_halfA(hbl, psb, lv)

                    rz = rzp.tile([128, 2 * H], bf16, tag="rz")
                    np_ = npp.tile([128, H], bf16, tag="np")
                    hnew = hnp.tile([128, H], bf16, tag="hnew")

                    # tail A (cols 0..512) while half B streams
                    emit_quarter(0, M, psb, xi, rz, np_, hnew)
                    emit_quarter(1, M, psb, xi, rz, np_, hnew)

                    # gate matmuls, half B: banks r1,n1,z1, with this level's
                    # ptA transposes interleaved after chunk 4 (tail A's
                    # h_new cols 0..512 are ready by then) so blend-A of the
                    # next level completes before half B ends
                    for nb in (1, 3, 5):
                        psb[nb] = pspool.tile([128, 512], f32, tag="ps", name=f"ps{nb}")
                    for c in range(KC):
                        lhsT = hbl[:, c * 128 : c * 128 + M]
                        for nb in (1, 3, 5):
                            nc.tensor.matmul(
                                psb[nb][:M, :],
                                lhsT,
                                whh[:, c * GW + nb * 512 : c * GW + (nb + 1) * 512],
                                start=(c == 0),
                                stop=(c == KC - 1),
                            )
                            if c == 0 and nb == 3:
                                nc.tensor.matmul(
                                    psb[nb][:M, :],
                                    ones1[0:1, :M],
                                    b2bc[0:1, 512:1024],
                                    start=False,
                                    stop=False,
                                )
                        if c == 4 and lv + 1 < NLEV:
                            ptA = pspool.tile([128, 512], bf16, tag="ps", name="ptA")
                            emit_pt(ptA, (0, 1, 2, 3), M, hnew)
                            emit_hist_copy(ptA, (0, 1, 2, 3), M, sl)
                            # blend-A of next level (DVE; deps now in hist)
                            hbl_cur = new_hbl(lv + 1)
                            emit_blend(nc.vector, hbl_cur, levels[lv + 1], 0)

                    # tail B (cols 512..1024) hides under next level's stream
                    emit_quarter(2, M, psb, xi, rz, np_, hnew)
                    emit_quarter(3, M, psb, xi, rz, np_, hnew)
                    if lv + 1 < NLEV:
                        pending = (hnew, M, sl)

                    if lv == lv_out:
                        i = idx_of[t_out]
                        outt = opool.tile([128, H], f32, tag="outt")
                        nc.vector.tensor_scalar_mul(outt[:M, :], hnew[:M, :], 2.0)
                        nc.sync.dma_start(out_d[:], outt[i * BL : (i + 1) * BL, :])

    nc.finalize()
    return nc


def kernel(**inputs):
    x = np.asarray(inputs["x"], np.float32)
    W_ih = np.asarray(inputs["W_ih"], np.float32)
    W_hh = np.asarray(inputs["W_hh"], np.float32)
    b_ih = np.asarray(inputs["b_ih"], np.float32)
    b_hh = np.asarray(inputs["b_hh"], np.float32)
    w1 = np.asarray(inputs["w1"], np.int32)
    w2 = np.asarray(inputs["w2"], np.int32)
    assert int(inputs["skip_size"]) == SKIP

    levels, order, lv_of, idx_of, d1, d2 = _plan(w1, w2)
    assert max(len(lv) for lv in levels) * BL <= 128
    nc = _build(levels, lv_of, idx_of, d1, d2)

    perm = np.concatenate([np.arange(0, H), np.arange(2 * H, G3), np.arange(H, 2 * H)])
    W_hh_p = W_hh[perm]
    W_ih_p = W_ih[perm]
    bias = (b_ih + b_hh).copy()
    bias[2 * H :] = b_ih[2 * H :]  # n-part: only b_ih (b_hh_n applied inside r*(.))
    bias = bias[perm]
    whh_t = np.ascontiguousarray(W_hh_p.T.reshape(KC, 128, G3)).astype(bfnp)
    # append a 128x128 identity block per K-chunk: streams h_blend through
    # the PE into psum in batch layout (replaces the hbb transpose DMAs)
    eye = np.broadcast_to(np.eye(128, dtype=bfnp), (KC, 128, 128))
    whh_t = np.concatenate([whh_t, eye], axis=2)
    wih_t = np.ascontiguousarray(W_ih_p.T.reshape(2, 128, G3)).astype(bfnp)
    biasg = np.broadcast_to(bias, (128, G3)).astype(np.float32).copy()
    bias2g = np.broadcast_to(b_hh[2 * H :], (128, H)).astype(bfnp).copy()
    in_maps = []
    for c in range(NCORES):
        xc = x[c * BL : (c + 1) * BL]  # [8, T, X]
        xsrt = xc[:, order, :]  # level-sorted
        xs = np.ascontiguousarray(xsrt.transpose(2, 1, 0).reshape(2, 128, T * BL))
        in_maps.append(
            {
                "xs": xs.astype(bfnp),
                "wih": wih_t,
                "whh": whh_t,
                "biasg": biasg,
                "bias2": bias2g,
                "ident": np.eye(128, dtype=bfnp),
            }
        )
    res = run_bass_kernel_spmd(nc, in_maps, core_ids=list(range(NCORES)))
    if getattr(res, "exec_time_ns", None):
        print("HW exec time:", res.exec_time_ns, "ns")
    global LAST_RESULT
    LAST_RESULT = res
    out = np.concatenate([res.results[c]["out"] for c in range(NCORES)], axis=0)
    return np.asarray(out, np.float32)


LAST_RESULT = None


if __name__ == "__main__":
    rng = np.random.default_rng(0)
    ins = {
        "x": rng.standard_normal((B, T, X)).astype(np.float32),
        "W_ih": rng.standard_normal((G3, X)).astype(np.float32) / 32,
        "W_hh": rng.standard_normal((G3, H)).astype(np.float32) / 32,
        "b_ih": rng.standard_normal(G3).astype(np.float32) / 32,
        "b_hh": rng.standard_normal(G3).astype(np.float32) / 32,
        "w1": rng.integers(0, 2, T).astype(np.int32),
        "w2": rng.integers(0, 2, T).astype(np.int32),
        "skip_size": 5,
    }
    ins["w2"] = np.where(ins["w1"] == 0, 1, ins["w2"]).astype(np.int32)
    out = kernel(**ins)
    print("ran", out.shape, out.dtype, float(np.abs(out).mean()))



# revision 7
# speedup vs baseline: 1.7788x; 1.7788x over previous
"""Trainium2 Bass kernel for nn_Encoder_60112362275055 (GRU with skip connections).

B=64, T=512, X=256, H=1024, skip_size=5. Output = 2 * h_{T-1}  -> [64, 1024].

Data-parallel over batch (8 cores x B_local=8), zero cross-core traffic.
The skip structure (w1[t]==0 drops the h_{t-1} dependency) turns the scan
into a DAG; only the ~318 ancestors of t=T-1 are computed (87 levels,
max width 8 -> M <= 64 rows per level per core).

v3 design (vs the v2 half-split baseline):
- 2x PE column-tiling: h-cols 0:512 of every per-node vector live on psum
  partitions [0:M] (array col-group 0) and h-cols 512:1024 on partitions
  [64:64+M] (col-group 64). The two groups stream different W_hh column
  slices CONCURRENTLY -> gate-matmul wall time halves.
- whole-level gate matmuls in bank-major order (r, n, z-half1, z-half2) so
  the r/n elementwise tail overlaps the z streams; xi_z is folded into the
  z psum via small identity matmuls to shorten the end-of-level chain.
- h_blend in batch layout comes from PE transposes of the blended
  stationary (hbl) -> bf16 psum, replacing the v2 identity passthrough.
- all matmul operands and the elementwise tail run in bf16 (psum f32 for
  gate accumulation, bf16 for transposes).
"""

import os
import sys

import numpy as np

sys.path.insert(0, "/opt/trn_rl_repo")

NLEV_CAP = int(os.environ.get("NLEV_CAP", "0"))  # 0 = all levels

import ml_dtypes

import concourse.bacc as bacc
import concourse.mybir as mybir
from concourse import tile
from concourse.bass_utils import run_bass_kernel_spmd

B, T, X, H = 64, 512, 256, 1024
SKIP = 5
NCORES = 8
BL = B // NCORES  # 8
G3 = 3 * H  # 3072
KC = H // 128  # 8 K-chunks
NLEVH = 16  # rolling history depth in levels (max dep distance ~10)
WCOL = 64  # node-cols per (level, chunk) hist region (max M=64)
HALF = H // 2  # 512

f32 = mybir.dt.float32
bf16 = mybir.dt.bfloat16
AF = mybir.ActivationFunctionType
bfnp = ml_dtypes.bfloat16


def _skip_plan(T, skip_size):
    slots = np.zeros(T, np.int32)
    use_zero = np.zeros(T, np.float32)
    for i in range(T):
        if i < skip_size:
            if 2 * i < skip_size:
                use_zero[i] = 1.0
            else:
                slots[i] = (skip_size - i) - 1
        else:
            if i - skip_size < skip_size:
                use_zero[i] = 1.0
            else:
                slots[i] = 2 * skip_size - 1
    return slots, use_zero


def _plan(w1, w2):
    """Topological levels over the ancestor set of t=T-1."""
    slots, use_zero = _skip_plan(T, SKIP)
    d1 = np.full(T, -1, np.int64)
    d2 = np.full(T, -1, np.int64)
    for t in range(T):
        if w1[t] == 1 and t - 1 >= 0:
            d1[t] = t - 1
        if w2[t] == 1 and use_zero[t] == 0.0 and t - 1 - slots[t] >= 0:
            d2[t] = t - 1 - slots[t]
    anc = set()
    stack = [T - 1]
    while stack:
        t = stack.pop()
        if t in anc:
            continue
        anc.add(t)
        for d in (d1[t], d2[t]):
            if d >= 0 and d not in anc:
                stack.append(int(d))
    lv_of = {}
    for t in sorted(anc):
        deps = [d for d in (d1[t], d2[t]) if d >= 0]
        lv_of[t] = 1 + max((lv_of[d] for d in deps), default=-1)
    nlev = max(lv_of.values()) + 1
    levels = [[] for _ in range(nlev)]
    idx_of = {}
    for t in sorted(anc):
        idx_of[t] = len(levels[lv_of[t]])
        levels[lv_of[t]].append(t)
    order = [t for lv in levels for t in lv]
    maxdist = 0
    for t in sorted(anc):
        for d in (d1[t], d2[t]):
            if d >= 0:
                maxdist = max(maxdist, int(lv_of[t] - lv_of[d]))
    assert maxdist + 3 <= NLEVH, (maxdist, "NLEVH too small")
    assert max(len(lv) for lv in levels) * BL <= 64
    assert lv_of[T - 1] == nlev - 1
    return levels, order, lv_of, idx_of, d1, d2


def _build(levels, lv_of, idx_of, d1, d2, order):
    nc = bacc.Bacc(None)

    NROW = len(order)
    MT = (NROW * BL + 127) // 128  # phase-1 M-tiles
    NPAD = MT * 128

    xs_d = nc.dram_tensor("xs", [2, 128, NPAD], bf16, kind="ExternalInput")
    wih_d = nc.dram_tensor("wih", [2, 128, G3], bf16, kind="ExternalInput")
    whh_d = nc.dram_tensor("whh", [KC, 128, G3], bf16, kind="ExternalInput")
    biasg_d = nc.dram_tensor("biasg", [128, G3], f32, kind="ExternalInput")
    bias2_d = nc.dram_tensor("bias2", [128, H], bf16, kind="ExternalInput")
    # ident cols 0:128 = I_128; 128:192 = [I64;0]; 192:256 = [0;I64]
    ident_d = nc.dram_tensor("ident", [128, 256], bf16, kind="ExternalInput")
    out_d = nc.dram_tensor("out", [BL, H], f32, kind="ExternalOutput")
    xi_d = nc.dram_tensor("xi_scratch", [NPAD, G3], bf16)

    NLEV = len(levels)
    if NLEV_CAP:
        NLEV = min(NLEV, NLEV_CAP)
        levels = levels[:NLEV]
    t_out = T - 1

    with tile.TileContext(nc) as tc:
        with (
            tc.tile_pool(name="rec", bufs=1) as rpool,
            tc.tile_pool(name="ps", bufs=8, space="PSUM") as pspool,
        ):
            whh = rpool.tile([128, KC * G3], bf16)  # 48 KB/partition
            nc.sync.dma_start(
                whh[:].rearrange("p (k f) -> p k f", k=KC),
                whh_d.rearrange("k p f -> p k f"),
            )
            b2bc = rpool.tile([128, H], bf16)
            nc.sync.dma_start(b2bc[:], bias2_d[:])
            ident = rpool.tile([128, 256], bf16)
            nc.sync.dma_start(ident[:], ident_d[:])
            # hist^T: [128, slot(16) x chunk(8) x node-col(64)] bf16
            hist = rpool.tile([128, NLEVH * KC * WCOL], bf16)
            hv = hist[:].rearrange("p (s c j) -> p s c j", s=NLEVH, c=KC)

            # ---------- phase 1: xi = x @ W_ih.T + bias (level-sorted rows) --
            with tc.tile_pool(name="ph1", bufs=1) as p1, tc.tile_pool(
                name="ph1b", bufs=3
            ) as p1b:
                wih = p1.tile([128, 2 * G3], bf16)
                nc.sync.dma_start(
                    wih[:].rearrange("p (k f) -> p k f", k=2),
                    wih_d.rearrange("k p f -> p k f"),
                )
                xs = p1.tile([128, 2 * NPAD], bf16)
                nc.sync.dma_start(
                    xs[:].rearrange("p (k f) -> p k f", k=2),
                    xs_d.rearrange("k p f -> p k f"),
                )
                biasg = p1.tile([128, G3], f32)
                nc.sync.dma_start(biasg[:], biasg_d[:])
                NB = G3 // 512
                for m in range(MT):
                    for nb in range(NB):
                        ps = pspool.tile([128, 512], f32, tag="ps")
                        for k in range(2):
                            nc.tensor.matmul(
                                ps[:],
                                xs[:, k * NPAD + m * 128 : k * NPAD + (m + 1) * 128],
                                wih[:, k * G3 + nb * 512 : k * G3 + (nb + 1) * 512],
                                start=(k == 0),
                                stop=(k == 1),
                            )
                        sb = p1b.tile([128, 512], bf16, tag="xsb")
                        nc.vector.tensor_add(
                            sb[:], ps[:], biasg[:, nb * 512 : (nb + 1) * 512]
                        )
                        nc.sync.dma_start(
                            xi_d[m * 128 : (m + 1) * 128, nb * 512 : (nb + 1) * 512],
                            sb[:],
                        )

            # ---------- phase 2: recurrence over DAG levels ----------
            with (
                tc.tile_pool(name="rzp", bufs=2) as rzp,
                tc.tile_pool(name="npp", bufs=2) as npp,
                tc.tile_pool(name="hbm", bufs=2) as hbmp,
                tc.tile_pool(name="hnp", bufs=3) as hnp,
                tc.tile_pool(name="outp", bufs=1) as opool,
            ):
                hbl_rot = [
                    rpool.tile([128, KC * WCOL], bf16, name=f"hblr{i}")
                    for i in range(3)
                ]
                xi_rot = [
                    rpool.tile([128, 3 * HALF], bf16, name=f"xir{i}")
                    for i in range(3)
                ]

                def hist_pair(t, c0):
                    """[128, 2, BL] view of dep t's h^T chunks c0, c0+1."""
                    sl = int(lv_of[t]) % NLEVH
                    j = idx_of[t]
                    return hv[:, sl, c0 : c0 + 2, j * BL : (j + 1) * BL]

                def emit_blend(eng, hbl, nodes, half):
                    """Blend deps into hbl chunks {0,1,4,5} (half 0) or
                    {2,3,6,7} (half 1); two 2-chunk ops per node."""
                    hblv = hbl[:].rearrange("p (c j) -> p c j", c=KC)
                    for c0 in ((0, 4) if half == 0 else (2, 6)):
                        for i, t in enumerate(nodes):
                            dst = hblv[:, c0 : c0 + 2, i * BL : (i + 1) * BL]
                            a, b2 = int(d1[t]), int(d2[t])
                            if a < 0 and b2 < 0:
                                eng.memset(dst, 0.0)
                            elif a >= 0 and b2 >= 0:
                                eng.tensor_add(
                                    dst, hist_pair(a, c0), hist_pair(b2, c0)
                                )
                            else:
                                eng.tensor_copy(
                                    dst, hist_pair(a if a >= 0 else b2, c0)
                                )

                # row offset of each level's xi rows (level-sorted, tight)
                srow = [0]
                for lvw in levels:
                    srow.append(srow[-1] + len(lvw))

                ones1 = rpool.tile([1, 128], bf16)
                nc.vector.memset(ones1[:], 1.0)

                # pre-loop: zero all rotation buffers (garbage lanes must
                # hold written data for the full-64-wide matmul reads)
                for tbuf in hbl_rot:
                    nc.vector.memset(tbuf[:, :], 0.0)
                for tbuf in xi_rot:
                    nc.gpsimd.memset(tbuf[:, :], 0.0)
                hbl_cur = hbl_rot[0]
                emit_blend(nc.vector, hbl_cur, levels[0], 0)
                emit_blend(nc.gpsimd, hbl_cur, levels[0], 1)

                # chunk stream order: h1-chunks first (their blend lands first)
                CORD = (0, 1, 4, 5, 2, 3, 6, 7)

                for lv, nodes in enumerate(levels):
                    w = len(nodes)
                    M = BL * w
                    sl = lv % NLEVH
                    hbl = hbl_cur

                    # xi rows for this level (SWDGE, off the HWDGE queues)
                    # [0:M] rows = gate cols 0:1536 (rA zA nA),
                    # [64:64+M]  = gate cols 1536:3072 (rB zB nB)
                    xi = xi_rot[lv % 3]
                    r0 = srow[lv] * BL
                    nc.gpsimd.dma_start(xi[0:M, :], xi_d[r0 : r0 + M, 0 : 3 * HALF])
                    nc.gpsimd.dma_start(
                        xi[64 : 64 + M, :], xi_d[r0 : r0 + M, 3 * HALF : G3]
                    )

                    ps_hb = pspool.tile([128, 512], f32, tag="ps", name="ps_hb")
                    ps_r = pspool.tile([128, 512], f32, tag="ps", name="ps_r")
                    ps_n = pspool.tile([128, 512], f32, tag="ps", name="ps_n")
                    ps_z = pspool.tile([128, 512], f32, tag="ps", name="ps_z")

                    def gate_stream(psb, gcol, lo=0, hi=512):
                        for ci, c in enumerate(CORD):
                            lhsT = hbl[:, c * WCOL : (c + 1) * WCOL]
                            st = ci == 0
                            sp = ci == KC - 1
                            nc.tensor.matmul(
                                psb[0:64, lo:hi],
                                lhsT,
                                whh[:, c * G3 + gcol * 512 + lo : c * G3 + gcol * 512 + hi],
                                start=st,
                                stop=sp,
                                skip_group_check=True,
                            )
                            nc.tensor.matmul(
                                psb[64:128, lo:hi],
                                lhsT,
                                whh[:, c * G3 + 1536 + gcol * 512 + lo : c * G3 + 1536 + gcol * 512 + hi],
                                start=st,
                                stop=sp,
                                skip_group_check=True,
                            )

                    # ---- r gate (bank-major, first) ----
                    gate_stream(ps_r, 0)
                    # ---- n gate + b_hh_n rank-1 fold ----
                    gate_stream(ps_n, 2)
                    nc.tensor.matmul(
                        ps_n[0:64, :], ones1[0:1, 0:64], b2bc[0:1, 0:512],
                        start=False, stop=False, skip_group_check=True,
                    )
                    nc.tensor.matmul(
                        ps_n[64:128, :], ones1[0:1, 0:64], b2bc[0:1, 512:1024],
                        start=False, stop=False, skip_group_check=True,
                        tile_position=(0, 64),
                    )

                    # ---- h_blend batch layout: PE transposes of hbl ----
                    # chunk c (A: 0-3 / B: 4-7) -> ps_hb rows [0:M]/[64:64+M]
                    for c in range(4):
                        nc.tensor.matmul(
                            ps_hb[0:64, c * 128 : (c + 1) * 128],
                            hbl[:, c * WCOL : (c + 1) * WCOL],
                            ident[:, 0:128],
                            start=True, stop=True, skip_group_check=True,
                        )
                        nc.tensor.matmul(
                            ps_hb[64:128, c * 128 : (c + 1) * 128],
                            hbl[:, (c + 4) * WCOL : (c + 5) * WCOL],
                            ident[:, 0:128],
                            start=True, stop=True, skip_group_check=True,
                        )

                    rz = rzp.tile([128, 2 * 512], bf16, tag="rz")
                    np_ = npp.tile([128, 512], bf16, tag="np")
                    hbmn = hbmp.tile([128, 512], bf16, tag="hbmn")
                    hnew = hnp.tile([128, 512], bf16, tag="hnew")

                    # r = sigmoid(ps_r + xi_r)  (overlaps z streams)
                    nc.vector.tensor_add(rz[:, 0:512], ps_r[:, :], xi[:, 0:512])
                    nc.scalar.activation(rz[:, 0:512], rz[:, 0:512], AF.Sigmoid)

                    # ---- z gate in two N-halves, xi_z folded via identity ----
                    for zh in range(2):
                        lo, hi = zh * 256, (zh + 1) * 256
                        gate_stream(ps_z, 1, lo, hi)
                        nc.tensor.matmul(
                            ps_z[0:64, lo:hi],
                            ident[:, 128:192],
                            xi[:, 512 + lo : 512 + hi],
                            start=False, stop=False, skip_group_check=True,
                        )
                        nc.tensor.matmul(
                            ps_z[64:128, lo:hi],
                            ident[:, 192:256],
                            xi[:, 512 + lo : 512 + hi],
                            start=False, stop=False, skip_group_check=True,
                        )

                    # n = tanh(xi_n + r * ps_n)
                    nc.vector.tensor_mul(np_[:, :], ps_n[:, :], rz[:, 0:512])
                    nc.vector.tensor_add(np_[:, :], np_[:, :], xi[:, 1024:1536])
                    nc.scalar.activation(np_[:, :], np_[:, :], AF.Tanh)
                    # hbmn = hb - n
                    nc.vector.tensor_sub(hbmn[:, :], ps_hb[:, :], np_[:, :])

                    # z = sigmoid(ps_z) (xi_z already in psum), in halves
                    nc.scalar.activation(rz[:, 512:768], ps_z[:, 0:256], AF.Sigmoid)
                    nc.scalar.activation(rz[:, 768:1024], ps_z[:, 256:512], AF.Sigmoid)

                    # h_new = n + z*(hb-n), in halves; each half's transposes,
                    # hist copies and next-level blends follow immediately
                    for hh in range(2):
                        cs = slice(hh * 256, (hh + 1) * 256)
                        nc.vector.tensor_mul(
                            hbmn[:, cs], hbmn[:, cs],
                            rz[:, 512 + hh * 256 : 512 + (hh + 1) * 256],
                        )
                        nc.vector.tensor_add(hnew[:, cs], hbmn[:, cs], np_[:, cs])

                        if lv + 1 < NLEV:
                            pt = pspool.tile([128, 256], f32, tag="ps", name="pt")
                            for k in range(2):
                                col = hh * 256 + k * 128
                                nc.tensor.matmul(
                                    pt[:, k * WCOL : (k + 1) * WCOL],
                                    hnew[:, col : col + 128],
                                    ident[:, 128:192],
                                    start=True, stop=True,
                                    skip_group_check=True,
                                )
                                nc.tensor.matmul(
                                    pt[:, (2 + k) * WCOL : (3 + k) * WCOL],
                                    hnew[:, col : col + 128],
                                    ident[:, 192:256],
                                    start=True, stop=True,
                                    skip_group_check=True,
                                )
                            # psum -> hist^T: chunks hh*2,hh*2+1 and +4,+5
                            base = sl * KC * WCOL
                            nc.scalar.copy(
                                hist[:, base + hh * 2 * WCOL : base + hh * 2 * WCOL + 128],
                                pt[:, 0:128],
                            )
                            nc.scalar.copy(
                                hist[:, base + (hh * 2 + 4) * WCOL : base + (hh * 2 + 4) * WCOL + 128],
                                pt[:, 128:256],
                            )
                            if hh == 0:
                                hbl_cur = hbl_rot[(lv + 1) % 3]
                            emit_blend(
                                nc.vector if hh == 0 else nc.gpsimd,
                                hbl_cur, levels[lv + 1], hh,
                            )

                    if lv == NLEV - 1:
                        j = idx_of[t_out] if not NLEV_CAP else 0
                        outt = opool.tile([128, HALF], f32, tag="outt")
                        nc.vector.tensor_scalar_mul(outt[:, :], hnew[:, :], 2.0)
                        nc.sync.dma_start(
                            out_d[:, 0:HALF], outt[j * BL : (j + 1) * BL, :]
                        )
                        nc.sync.dma_start(
                            out_d[:, HALF:H], outt[64 + j * BL : 64 + (j + 1) * BL, :]
                        )

    nc.finalize()
    return nc


def kernel(**inputs):
    x = np.asarray(inputs["x"], np.float32)
    W_ih = np.asarray(inputs["W_ih"], np.float32)
    W_hh = np.asarray(inputs["W_hh"], np.float32)
    b_ih = np.asarray(inputs["b_ih"], np.float32)
    b_hh = np.asarray(inputs["b_hh"], np.float32)
    w1 = np.asarray(inputs["w1"], np.int32)
    w2 = np.asarray(inputs["w2"], np.int32)
    assert int(inputs["skip_size"]) == SKIP

    levels, order, lv_of, idx_of, d1, d2 = _plan(w1, w2)
    nc = _build(levels, lv_of, idx_of, d1, d2, order)

    NROW = len(order)
    MT = (NROW * BL + 127) // 128
    NPAD = MT * 128

    # gate-column permutation: [rA zA nA rB zB nB], A = cols 0:512 of a gate
    perm = np.concatenate(
        [
            np.arange(0, HALF),
            np.arange(H, H + HALF),
            np.arange(2 * H, 2 * H + HALF),
            np.arange(HALF, H),
            np.arange(H + HALF, 2 * H),
            np.arange(2 * H + HALF, 3 * H),
        ]
    )
    W_hh_p = W_hh[perm]
    W_ih_p = W_ih[perm]
    bias = (b_ih + b_hh).copy()
    bias[2 * H :] = b_ih[2 * H :]  # n-part: only b_ih (b_hh_n inside r*(.))
    bias = bias[perm]
    whh_t = np.ascontiguousarray(W_hh_p.T.reshape(KC, 128, G3)).astype(bfnp)
    wih_t = np.ascontiguousarray(W_ih_p.T.reshape(2, 128, G3)).astype(bfnp)
    biasg = np.broadcast_to(bias, (128, G3)).astype(np.float32).copy()
    # b_hh_n: cols 0:512 = A-half, 512:1024 = B-half
    bias2g = np.broadcast_to(b_hh[2 * H :], (128, H)).astype(bfnp).copy()
    identh = np.zeros((128, 256), dtype=bfnp)
    identh[:, 0:128] = np.eye(128, dtype=bfnp)
    identh[0:64, 128:192] = np.eye(64, dtype=bfnp)
    identh[64:128, 192:256] = np.eye(64, dtype=bfnp)
    in_maps = []
    for c in range(NCORES):
        xc = x[c * BL : (c + 1) * BL]  # [8, T, X]
        xsrt = xc[:, order, :]  # pruned, level-sorted: [8, NROW, 256]
        xs = xsrt.transpose(2, 1, 0).reshape(2, 128, NROW * BL)
        xsp = np.zeros((2, 128, NPAD), np.float32)
        xsp[:, :, : NROW * BL] = xs
        in_maps.append(
            {
                "xs": xsp.astype(bfnp),
                "wih": wih_t,
                "whh": whh_t,
                "biasg": biasg,
                "bias2": bias2g,
                "ident": identh,
            }
        )
    res = run_bass_kernel_spmd(nc, in_maps, core_ids=list(range(NCORES)))
    if getattr(res, "exec_time_ns", None):
        print("HW exec time:", res.exec_time_ns, "ns")
    global LAST_RESULT
    LAST_RESULT = res
    out = np.concatenate([res.results[c]["out"] for c in range(NCORES)], axis=0)
    return np.asarray(out, np.float32)


LAST_RESULT = None
